# revision 6
# baseline (speedup 1.0000x reference)
"""2D Gaussian splat rasterizer on 8 Trainium2 NeuronCores.

Math: for gaussian n, pixel (x,y):
    quad'(n,p) = -0.5 * d^T Sigma^-1 d + log(opacity_n * norm_n)
is a degree-2 polynomial in (x, y), i.e. a rank-6 bilinear form
    quad'[n, p] = K[n, :] @ F[:, p],   F = [x'^2, x'y', y'^2, x', y', 1]
with tile-local pixel coords x', y' (origin shift folded into K per tile
for fp32 accuracy).  Device pipeline per 512-pixel tile:
    PE  : quad = K6^T @ F          (K=6 matmul, out 128 gauss x 512 pix)
    ACT : G = exp(quad)            (PSUM -> SBUF)
    PE  : out4 += [colors|1]^T @ G (K=128 matmul, accumulated over 4
                                    gaussian blocks -> RGB sums + weight)
    DVE : normalize image = colorsum * recip(max(wsum, 1e-8))
Canvas rows are sharded across the 8 cores (32 rows each, no collectives).
"""
import numpy as np

H, W, C, N = 256, 256, 3, 512
NCORES = 8
ROWS = H // NCORES            # 32 canvas rows per core
TR, TC = 4, 4                 # tile grid per core: 4x4 tiles of 8x64 px
TY, TX = ROWS // TR, W // TC  # tile = 8 rows x 64 cols = 512 pixels
PIX = TY * TX                 # 512 pixels per tile
NTILES = TR * TC              # 16 tiles per core
NBLK = N // 128               # 4 gaussian blocks of 128

_CACHE = {}


def _install_walrus_workarounds():
    """This walrus build allows only ONE sync wait per instruction.

    1) TileContext's exit Drain normally carries one wait per outstanding
       semaphore -> pre-emit single-wait SP nops and give the Drain a
       satisfied clock.
    2) Any scheduled instruction may still get 2+ waits -> post-process
       the serialized BIR: hoist extra waits onto single-wait NoOps
       inserted directly before the instruction on the same engine.
    """
    import json as _json
    import concourse.tile as tile_mod
    import concourse.bass as bass_mod
    from concourse.vector_clock import ScopedClock

    if getattr(bass_mod.Bass, "_gs2d_patched", False):
        return

    def _patched_drain_and_barrier(self, tick_clock, wait_clock):
        nc = self.nc
        vec = tick_clock.global_clock
        for proc in range(len(vec)):
            tick = vec[proc]
            if tick <= 0:
                continue
            single = ScopedClock()
            single.require_at_least(None, proc, tick)
            nop = nc.sync.nop(nofuse=True, hint="drain_split_wait")
            wait_clock.add_sem_waits(nop.ins, single)
        full = ScopedClock({None: vec.copy()})
        cur = ScopedClock({None: vec.copy()})
        drain_inst = nc.sync.drain()
        wait_clock.add_sem_waits(drain_inst.ins, full, cur)
        nc.all_engine_barrier()
        assert self.sems is not None
        popped = nc._tile_sem_poison_stack.pop()
        assert popped is self._sem_poison
        nc.clear_and_free_semaphores(list(self.sems.allocated().values()))
        nc.all_engine_barrier()

    tile_mod.TileContext._drain_and_barrier = _patched_drain_and_barrier

    _orig_to_json_bytes = bass_mod.Bass.to_json_bytes
    ctr = [7000000]

    def _split_multiwait(raw):
        m = _json.loads(raw)
        changed_any = False
        for f in m.get("functions", []):
            for bb in f.get("blocks", []):
                insts = bb.get("instructions")
                if not insts:
                    continue
                out, changed = [], False
                for ins in insts:
                    si = ins.get("sync_info")
                    ow = (si or {}).get("on_wait") or []
                    if len(ow) > 1:
                        changed = True
                        for wt in ow[:-1]:
                            ctr[0] += 1
                            out.append({
                                "debug": ins.get("debug", 0),
                                "engine": ins["engine"],
                                "ins": [],
                                "name": "I-%d" % ctr[0],
                                "opcode": "NoOp",
                                "outs": [],
                                "sync_info": {"on_update": [], "on_wait": [wt]},
                                "text_hint": "split_wait",
                            })
                        si["on_wait"] = [ow[-1]]
                    out.append(ins)
                if changed:
                    bb["instructions"] = out
                    changed_any = True
        if not changed_any:
            return raw
        return _json.dumps(m).encode()

    def _patched_to_json_bytes(self):
        return _split_multiwait(_orig_to_json_bytes(self))

    bass_mod.Bass.to_json_bytes = _patched_to_json_bytes
    bass_mod.Bass._gs2d_patched = True


def _build_nc():
    import concourse.bass as bass
    import concourse.mybir as mybir
    import concourse.tile as tile

    f32 = mybir.dt.float32
    nc = bass.Bass()
    kbig = nc.dram_tensor("kbig", (6, NTILES * N), f32, kind="ExternalInput")
    feat = nc.dram_tensor("feat", (6, PIX), f32, kind="ExternalInput")
    colaug = nc.dram_tensor("colaug", (N, 4), f32, kind="ExternalInput")
    img = nc.dram_tensor("img", (ROWS, W, C), f32, kind="ExternalOutput")

    with tile.TileContext(nc) as tc:
        with (
            tc.tile_pool(name="singles", bufs=1) as singles,
            tc.tile_pool(name="gpool", bufs=4) as gpool,
            tc.tile_pool(name="qpool", bufs=3, space="PSUM") as qpool,
            tc.tile_pool(name="opool", bufs=2, space="PSUM") as opool,
            tc.tile_pool(name="tail", bufs=1) as tail,
        ):
            kb = singles.tile([6, NTILES * N], f32)
            nc.sync.dma_start(out=kb, in_=kbig[:, :])
            ft = singles.tile([6, PIX], f32)
            nc.sync.dma_start(out=ft, in_=feat[:, :])
            caug = singles.tile([128, 4 * NBLK], f32)
            for ni in range(NBLK):
                nc.sync.dma_start(
                    out=caug[:, 4 * ni:4 * ni + 4],
                    in_=colaug[128 * ni:128 * (ni + 1), :],
                )
            acc4 = singles.tile([4, NTILES * PIX], f32)

            for pt in range(NTILES):
                gs = []
                for h in range(2):
                    q = qpool.tile([128, 2 * PIX], f32, tag="quad")
                    for j in range(2):
                        ni = 2 * h + j
                        nc.tensor.matmul(
                            out=q[:, j * PIX:(j + 1) * PIX],
                            lhsT=kb[:, pt * N + ni * 128: pt * N + (ni + 1) * 128],
                            rhs=ft,
                            start=True, stop=True,
                        )
                    g = gpool.tile([128, 2 * PIX], f32, tag="g")
                    nc.scalar.activation(
                        out=g, in_=q, func=mybir.ActivationFunctionType.Exp)
                    gs.append(g)
                out4 = opool.tile([4, PIX], f32, tag="out4")
                for ni in range(NBLK):
                    nc.tensor.matmul(
                        out=out4,
                        lhsT=caug[:, 4 * ni:4 * ni + 4],
                        rhs=gs[ni // 2][:, (ni % 2) * PIX:(ni % 2 + 1) * PIX],
                        start=(ni == 0), stop=(ni == NBLK - 1),
                    )
                nc.vector.tensor_copy(
                    acc4[:, pt * PIX:(pt + 1) * PIX], out4)

            # tail: rearrange each channel plane to (128 part x 64), then
            # normalize and interleave RGB into the output staging tile.
            planes = [tail.tile([128, TX], f32, tag="pl%d" % ch,
                                name="plane%d" % ch) for ch in range(4)]
            for ch in range(4):
                # partitions q = 32*tr + 8*tc + yp <- acc4 free order
                # (tr, tc, yp, xp) is exactly contiguous: plain reshape.
                src = acc4[ch:ch + 1, :].rearrange("p (q xp) -> p q xp", xp=TX)
                nc.sync.dma_start(out=planes[ch], in_=src)
            wrec = planes[3]
            nc.vector.tensor_scalar(
                out=wrec, in0=wrec, scalar1=1e-8, scalar2=None,
                op0=mybir.AluOpType.max)
            nc.vector.reciprocal(out=wrec, in_=wrec)
            stage = tail.tile([128, TX * C], f32, tag="stage")
            for ch in range(C):
                nc.vector.tensor_mul(
                    out=stage[:, ch:TX * C:C], in0=planes[ch], in1=wrec)
            for tr in range(TR):
                # stage partitions (tc, yp) -> img rows 8*tr + yp, cols 64*tc
                nc.sync.dma_start(
                    out=img[TY * tr:TY * (tr + 1)].rearrange(
                        "yp (tc xp) c -> tc yp (xp c)", tc=TC, xp=TX),
                    in_=stage[32 * tr:32 * (tr + 1), :],
                )
    return nc


def _host_prep(means, covariances, colors, opacities):
    mx = means[:, 0].astype(np.float64)
    my = means[:, 1].astype(np.float64)
    cov = covariances.astype(np.float64)
    a, b, c = cov[:, 0, 0], cov[:, 0, 1], cov[:, 1, 1]
    det = a * c - b * b
    Ai, Bi, Ci = c / det, -b / det, a / det          # Sigma^-1 entries
    norm = 1.0 / (2.0 * np.pi * np.sqrt(det + 1e-8))
    with np.errstate(divide="ignore"):
        logw = np.log(opacities.astype(np.float64) * norm)
    logw = np.maximum(logw, -1e4)

    kbigs = []
    for core in range(NCORES):
        kb = np.empty((6, NTILES * N), dtype=np.float32)
        for tr in range(TR):
            for tc_ in range(TC):
                t = tr * TC + tc_
                x0, y0 = TX * tc_, ROWS * core + TY * tr
                u, v = mx - x0, my - y0
                k6 = np.stack([
                    -0.5 * Ai,
                    -Bi,
                    -0.5 * Ci,
                    Ai * u + Bi * v,
                    Ci * v + Bi * u,
                    -0.5 * (Ai * u * u + 2 * Bi * u * v + Ci * v * v) + logw,
                ])
                kb[:, t * N:(t + 1) * N] = k6.astype(np.float32)
        kbigs.append(kb)

    p = np.arange(PIX)
    yp, xp = (p // TX).astype(np.float64), (p % TX).astype(np.float64)
    feat = np.stack([xp * xp, xp * yp, yp * yp, xp, yp, np.ones(PIX)])
    feat = feat.astype(np.float32)

    colaug = np.concatenate(
        [colors.astype(np.float32), np.ones((N, 1), np.float32)], axis=1)
    return kbigs, feat, colaug


def kernel(means, covariances, colors, opacities, height, width, **_unused):
    assert int(height) == H and int(width) == W
    _install_walrus_workarounds()
    from concourse.bass_utils import run_bass_kernel_spmd

    if "nc" not in _CACHE:
        _CACHE["nc"] = _build_nc()
    nc = _CACHE["nc"]

    kbigs, feat, colaug = _host_prep(
        np.asarray(means), np.asarray(covariances),
        np.asarray(colors), np.asarray(opacities))
    in_maps = [{"kbig": kbigs[k], "feat": feat, "colaug": colaug}
               for k in range(NCORES)]
    res = run_bass_kernel_spmd(nc, in_maps, core_ids=list(range(NCORES)))
    out = np.concatenate([res.results[k]["img"] for k in range(NCORES)], axis=0)
    return out.astype(np.float32)
